# revision 3
# baseline (speedup 1.0000x reference)
"""Llama-3-8B-style GQA attention layer (bsz=1, seq=2048, dim=4096) on 8 TRN2 NeuronCores.

v2: pipelined quarter-granularity structure to hide the AllGather (the v1
bottleneck: 2x 8MB AGs cost ~205us/rep, barely overlapped).

Tensor-parallel over heads: core i owns Q heads 4i..4i+3 and KV head i.
Per rep, for each q-block qb (512 positions):
  A(qb):    QKV projections (transposed layout) + RoPE + V transpose.
            x loads prefetched during the previous attention block.
  attn(qb): causal attention for 4 heads; k-tile pairs share one psum+exp;
            diagonal tiles packed 2 per exp; denominator via DVE accumulate
            + rank-1 PE matmul; gpsimd partition-broadcast of 1/den.
            The wo GEMM for quarter qb-1 (128 MMs) is interleaved at head
            boundaries so PE fills the ACT-bound gaps; attn0 absorbs
            quarter 3 of the previous rep.
  AG(qb):   AllGather of this quarter's normalized O^T [512 -> 4096, 512]
            bf16 on the SP queue (dedicated, so the 4 AGs pipeline with all
            compute); the ag->SBUF loads for the wo GEMM queue right behind
            their AG on SP so no compute queue ever blocks on the AG sem.
Output: core i computes out[:, 512i:512(i+1)]; host concatenates.
"""
import numpy as np
import ml_dtypes

BF16 = ml_dtypes.bfloat16
N_CORES = 8
SEQ = 2048
DIM = 4096
HD = 128          # head dim
NQH = 4           # Q heads per core
QCOLS = NQH * HD  # 512
SM_SCALE = 1.0 / float(np.sqrt(HD))

_cache = {}


def _build_nc(reps: int = 1, stages: str = "ABCD"):
    import concourse.bacc as bacc
    import concourse.mybir as mybir
    import concourse.tile as tile
    import concourse.masks as masks

    dt = mybir.dt
    Alu = mybir.AluOpType
    Act = mybir.ActivationFunctionType

    fake_ag = "F" in stages
    do_d = ("D" in stages) and ("B" in stages) and ("C" in stages)

    nc = bacc.Bacc("TRN2", target_bir_lowering=False, debug=False)

    xT_e = nc.declare_dram_parameter("xT", [DIM, SEQ], dt.bfloat16, isOutput=False)
    wq_e = nc.declare_dram_parameter("wq", [DIM, QCOLS], dt.bfloat16, isOutput=False)
    wk_e = nc.declare_dram_parameter("wk", [DIM, HD], dt.bfloat16, isOutput=False)
    wv_e = nc.declare_dram_parameter("wv", [DIM, HD], dt.bfloat16, isOutput=False)
    wo_e = nc.declare_dram_parameter("wo", [DIM, QCOLS], dt.bfloat16, isOutput=False)
    cs_e = nc.declare_dram_parameter("cs", [256, SEQ], dt.bfloat16, isOutput=False)
    out_e = nc.declare_dram_parameter("out", [SEQ, QCOLS], dt.float32, isOutput=True)

    NSB = SEQ // 512   # 4 seq blocks of 512
    NCH = DIM // 128   # 32 contraction chunks
    NG = NCH // 4      # 4-chunk DMA groups

    ags = [nc.dram_tensor(f"ag{q}", [DIM, 512], dt.bfloat16, addr_space="Shared")
           for q in range(NSB)]

    with tile.TileContext(nc) as tc:
        with (
            tc.tile_pool(name="persist", bufs=1) as pp,
            tc.tile_pool(name="dram", bufs=1, space="DRAM") as dramp,
            tc.tile_pool(name="xtp", bufs=7) as xtp,
            tc.tile_pool(name="qbfp", bufs=3) as qbfp,
            tc.tile_pool(name="trp", bufs=2) as trp,
            tc.tile_pool(name="vtmp", bufs=2) as vtp,
            tc.tile_pool(name="ptp", bufs=3) as ptp,
            tc.tile_pool(name="daccp", bufs=2) as daccp,
            tc.tile_pool(name="denp", bufs=2) as denp,
            tc.tile_pool(name="oTp", bufs=3) as oTp,
            tc.tile_pool(name="atp", bufs=9) as atp,
            tc.tile_pool(name="dsbp", bufs=2) as dsbp,
        ):
            # ---- persistent SBUF tensors ----
            wq_g = [pp.tile([128, 4 * QCOLS], dt.bfloat16, name=f"wqg{g}") for g in range(NG)]
            wk_g = [pp.tile([128, 4 * HD], dt.bfloat16, name=f"wkg{g}") for g in range(NG)]
            wv_g = [pp.tile([128, 4 * HD], dt.bfloat16, name=f"wvg{g}") for g in range(NG)]
            wo_g = [pp.tile([128, 4 * QCOLS], dt.bfloat16, name=f"wog{g}") for g in range(NG)]
            wq_sb = [wq_g[c // 4][:, (c % 4) * QCOLS:(c % 4 + 1) * QCOLS] for c in range(NCH)]
            wk_sb = [wk_g[c // 4][:, (c % 4) * HD:(c % 4 + 1) * HD] for c in range(NCH)]
            wv_sb = [wv_g[c // 4][:, (c % 4) * HD:(c % 4 + 1) * HD] for c in range(NCH)]
            wo_sb = [wo_g[c // 4][:, (c % 4) * QCOLS:(c % 4 + 1) * QCOLS] for c in range(NCH)]
            cos_sb = pp.tile([128, SEQ], dt.bfloat16)
            sin_sb = pp.tile([128, SEQ], dt.bfloat16)
            tri01 = pp.tile([128, 128], dt.bfloat16)      # 1 iff k <= q
            ident = pp.tile([128, 128], dt.bfloat16)
            ones_col = pp.tile([128, 1], dt.bfloat16)
            ones_row = pp.tile([1, 128], dt.bfloat16)     # lhsT for 1/den broadcast
            qrope = [pp.tile([128, SEQ], dt.bfloat16, name=f"qrope{h}") for h in range(NQH)]
            krope = pp.tile([128, SEQ], dt.bfloat16)
            v_sb = pp.tile([128, SEQ], dt.bfloat16)

            agins = [dramp.tile([QCOLS, 512], dt.bfloat16, name=f"agin{q}")
                     for q in range(NSB)]

            for g in range(NG):
                gsl = slice(g * 512, (g + 1) * 512)
                nc.scalar.dma_start(wq_g[g][:].rearrange("p (c m) -> p c m", c=4),
                                    wq_e.ap()[gsl, :].rearrange("(c p) m -> p c m", p=128))
                nc.scalar.dma_start(wk_g[g][:].rearrange("p (c m) -> p c m", c=4),
                                    wk_e.ap()[gsl, :].rearrange("(c p) m -> p c m", p=128))
                nc.scalar.dma_start(wv_g[g][:].rearrange("p (c m) -> p c m", c=4),
                                    wv_e.ap()[gsl, :].rearrange("(c p) m -> p c m", p=128))
                if g == 0:
                    nc.scalar.dma_start(cos_sb[:], cs_e.ap()[0:128, :])
                    nc.scalar.dma_start(sin_sb[:], cs_e.ap()[128:256, :])

            nc.gpsimd.memset(tri01[:], 1.0)
            nc.gpsimd.affine_select(
                out=tri01[:], in_=tri01[:], compare_op=Alu.is_ge, fill=0.0,
                base=0, pattern=[[1, 128]], channel_multiplier=-1,
            )
            masks.make_identity(nc, ident[:])
            nc.gpsimd.memset(ones_col[:], 1.0)
            nc.gpsimd.memset(ones_row[:], 1.0)

            # ---------- helpers ----------
            xt_pending = {}  # sb -> list of 8 xt4 tiles (some loads maybe deferred)

            def emit_x_load(sb, g):
                sl = slice(sb * 512, (sb + 1) * 512)
                xt4 = xtp.tile([128, 4 * 512], dt.bfloat16, name="xt4")
                nc.scalar.dma_start(
                    xt4[:].rearrange("p (c s) -> p c s", c=4),
                    xT_e.ap()[g * 512:(g + 1) * 512, sl].rearrange(
                        "(c p) s -> p c s", p=128))
                return xt4

            def prefetch_x(sb, n):
                if sb not in xt_pending:
                    xt_pending[sb] = []
                while len(xt_pending[sb]) < n:
                    xt_pending[sb].append(emit_x_load(sb, len(xt_pending[sb])))

            def emit_ag(q):
                """AllGather of quarter q (agins[q] [512,512] -> ags[q]) + the
                SBUF loads for wo-GEMM pass 0 queued right behind it on SP."""
                if "C" not in stages:
                    return
                if fake_ag:
                    nc.scalar.dma_start(ags[q][0:QCOLS, :], agins[q][:])
                else:
                    nc.gpsimd.collective_compute(
                        "AllGather",
                        mybir.AluOpType.bypass,
                        replica_groups=[list(range(N_CORES))],
                        ins=[agins[q].opt()],
                        outs=[ags[q][:]],
                    )

            def load_ag_pass(qq, p, eng):
                """Load ag quarter qq, seq half p (256 cols) as 8 4-chunk tiles."""
                ats = []
                for pg in range(8):
                    at = atp.tile([128, 1024], dt.bfloat16, name="at")
                    eng.dma_start(
                        at[:].rearrange("p (c s) -> p c s", c=4),
                        ags[qq][pg * 512:(pg + 1) * 512,
                                p * 256:(p + 1) * 256].rearrange(
                                    "(c p) s -> p c s", p=128))
                    ats.append(at)
                return ats

            def make_d_quarter(qq):
                """wo GEMM for quarter qq as 4 closures (consumed at the head
                boundaries of the next attention block). Pass p computes out
                rows [qq*512+p*256, +256) via 2 strip psums."""
                state = {}

                def chunk(p, s, psW):
                    ats = state.get(p)
                    if ats is None:
                        ats = load_ag_pass(qq, p, nc.sync)
                        state[p] = ats
                    wops = psW.tile([128, 512], dt.float32, name="wops")
                    for pg in range(8):
                        at = ats[pg]
                        for cc in range(4):
                            c = 4 * pg + cc
                            nc.tensor.matmul(
                                wops[:],
                                at[:, cc * 256 + s * 128:cc * 256 + (s + 1) * 128],
                                wo_sb[c][:], start=(c == 0), stop=(c == NCH - 1))
                    dsb = dsbp.tile([128, 512], dt.float32, name="dsb")
                    nc.vector.tensor_copy(dsb[:], wops[:])
                    row0 = qq * 512 + p * 256 + s * 128
                    nc.scalar.dma_start(out_e.ap()[row0:row0 + 128, :], dsb[:])

                return [lambda psW, p=p, s=s: chunk(p, s, psW)
                        for p in range(2) for s in range(2)]

            def emit_a_block(sb):
                """QKV + RoPE + V-transpose for seq block sb."""
                sl = slice(sb * 512, (sb + 1) * 512)
                prefetch_x(sb, 2)  # ensure at least 2 loads issued
                with (
                    tc.tile_pool(name="psA", bufs=1, space="PSUM") as psA,
                    tc.tile_pool(name="psAT", bufs=2, space="PSUM") as psAT,
                ):
                    qps = [psA.tile([128, 512], dt.float32, name=f"qps{m}") for m in range(NQH)]
                    kps = psA.tile([128, 512], dt.float32, name="kps")
                    vps = psA.tile([128, 512], dt.float32, name="vps")
                    for g in range(NG):
                        prefetch_x(sb, min(g + 4, NG))
                        xt4 = xt_pending[sb][g]
                        for cc in range(4):
                            c = g * 4 + cc
                            xt = xt4[:, cc * 512:(cc + 1) * 512]
                            st, sp = (c == 0), (c == NCH - 1)
                            for m in range(NQH):
                                nc.tensor.matmul(qps[m][:], wq_sb[c][:, m * 128:(m + 1) * 128],
                                                 xt, start=st, stop=sp)
                            nc.tensor.matmul(kps[:], wk_sb[c][:], xt, start=st, stop=sp)
                            nc.tensor.matmul(vps[:], wv_sb[c][:], xt, start=st, stop=sp)
                    del xt_pending[sb]

                    # RoPE in bf16 on DVE
                    for h in range(NQH + 1):
                        ps = qps[h] if h < NQH else kps
                        dst = qrope[h] if h < NQH else krope
                        qbf = qbfp.tile([128, 512], dt.bfloat16, name="qbf")
                        nc.scalar.copy(qbf[:], ps[:])
                        tr_c = trp.tile([64, 512], dt.bfloat16, name="tr_c")
                        ti_s = trp.tile([64, 512], dt.bfloat16, name="ti_s")
                        nc.vector.tensor_mul(tr_c[:], qbf[0:64, :], cos_sb[0:64, sl])
                        nc.vector.tensor_mul(ti_s[:], qbf[64:128, :], sin_sb[64:128, sl])
                        nc.vector.tensor_sub(dst[0:64, sl], tr_c[:], ti_s[:])
                        tr_s = trp.tile([64, 512], dt.bfloat16, name="tr_c")
                        ti_c = trp.tile([64, 512], dt.bfloat16, name="ti_s")
                        nc.vector.tensor_mul(tr_s[:], qbf[0:64, :], sin_sb[0:64, sl])
                        nc.vector.tensor_mul(ti_c[:], qbf[64:128, :], cos_sb[64:128, sl])
                        nc.vector.tensor_add(dst[64:128, sl], tr_s[:], ti_c[:])

                    vT_sb = vtp.tile([128, 512], dt.bfloat16, name="vT_sb")
                    nc.scalar.copy(vT_sb[:], vps[:])
                    for t in range(4):
                        kt = sb * 4 + t
                        vtp_ps = psAT.tile([128, 128], dt.bfloat16, name="vtp_ps")
                        nc.tensor.transpose(vtp_ps[:], vT_sb[:, t * 128:(t + 1) * 128], ident[:])
                        nc.scalar.copy(v_sb[:, kt * HD:(kt + 1) * HD], vtp_ps[:])

            def emit_attn_block(qb, dq, next_sb, psS, psO, psDB, psW):
                """Attention for q-block qb; dq: pending wo-GEMM chunk closures."""
                qsl = slice(qb * 512, (qb + 1) * 512)
                n_pair = 2 * qb
                finish = [None]  # deferred normalize of the previous head

                def emit_finish():
                    if finish[0] is not None:
                        finish[0]()
                        finish[0] = None

                if dq:
                    dq.pop(0)(psW)  # fills the PE gap while RoPE drains on DVE
                for h in range(NQH):
                    # prefetch x loads for the next A block (up to 6 during attn)
                    if next_sb is not None:
                        prefetch_x(next_sb, min(2 + 2 * h, 6))
                    ops = psO.tile([128, 512], dt.float32, name="ops")
                    dacc = daccp.tile([128, 1024], dt.bfloat16, name="dacc")
                    gi = 0  # psS-group index within this head
                    for u in range(n_pair):
                        kt = 2 * u
                        sps = psS.tile([128, 1024], dt.float32, name="sps")
                        nc.tensor.matmul(sps[:, 0:512], krope[:, kt * 128:(kt + 1) * 128],
                                         qrope[h][:, qsl], start=True, stop=True)
                        nc.tensor.matmul(sps[:, 512:1024],
                                         krope[:, (kt + 1) * 128:(kt + 2) * 128],
                                         qrope[h][:, qsl], start=True, stop=True)
                        if gi == 0:
                            emit_finish()  # previous head's normalize, PE-covered
                        pt = ptp.tile([128, 1024], dt.bfloat16, name="pt")
                        nc.scalar.activation(pt[:], sps[:], Act.Exp, scale=SM_SCALE)
                        nc.tensor.matmul(ops[:], v_sb[:, kt * HD:(kt + 1) * HD],
                                         pt[:, 0:512], start=(kt == 0), stop=False,
                                         skip_group_check=True)
                        nc.tensor.matmul(ops[:], v_sb[:, (kt + 1) * HD:(kt + 2) * HD],
                                         pt[:, 512:1024], start=False, stop=False,
                                         skip_group_check=True)
                        if u == 0:
                            nc.vector.tensor_copy(dacc[:], pt[:])
                        else:
                            nc.vector.tensor_add(dacc[:], dacc[:], pt[:])
                        gi += 1
                        if gi == 2 and dq and h <= 1:
                            dq.pop(0)(psW)

                    # 4 diagonal tiles, packed 2 per psum/exp
                    kt0 = 4 * qb
                    for d2 in range(2):
                        ka = kt0 + 2 * d2
                        wa = 512 - 128 * (2 * d2)       # 512 or 256
                        wb = 512 - 128 * (2 * d2 + 1)   # 384 or 128
                        oa = 512 - wa
                        ob = 512 - wb
                        sps = psS.tile([128, 1024], dt.float32, name="sps")
                        nc.tensor.matmul(sps[:, 0:wa], krope[:, ka * 128:(ka + 1) * 128],
                                         qrope[h][:, qb * 512 + oa:(qb + 1) * 512],
                                         start=True, stop=True)
                        nc.tensor.matmul(sps[:, wa:wa + wb],
                                         krope[:, (ka + 1) * 128:(ka + 2) * 128],
                                         qrope[h][:, qb * 512 + ob:(qb + 1) * 512],
                                         start=True, stop=True)
                        if gi == 0:
                            emit_finish()
                        pt = ptp.tile([128, 1024], dt.bfloat16, name="pt")
                        nc.scalar.activation(pt[:, 0:wa + wb], sps[:, 0:wa + wb],
                                             Act.Exp, scale=SM_SCALE)
                        nc.vector.tensor_mul(pt[:, 0:128], pt[:, 0:128], tri01[:])
                        nc.vector.tensor_mul(pt[:, wa:wa + 128], pt[:, wa:wa + 128], tri01[:])
                        nc.tensor.matmul(ops[:, oa:512], v_sb[:, ka * HD:(ka + 1) * HD],
                                         pt[:, 0:wa], start=(ka == 0), stop=False,
                                         skip_group_check=True)
                        nc.tensor.matmul(ops[:, ob:512],
                                         v_sb[:, (ka + 1) * HD:(ka + 2) * HD],
                                         pt[:, wa:wa + wb], start=False,
                                         stop=(d2 == 1), skip_group_check=True)
                        if qb == 0 and d2 == 0:
                            nc.vector.tensor_copy(dacc[:, 0:512], pt[:, 0:512])
                        else:
                            nc.vector.tensor_add(dacc[:, oa:512], dacc[:, oa:512],
                                                 pt[:, 0:wa])
                        nc.vector.tensor_add(dacc[:, ob:512], dacc[:, ob:512],
                                             pt[:, wa:wa + wb])
                        gi += 1
                        if gi == 2 and dq and h <= 1:
                            dq.pop(0)(psW)

                    # denominator: rank-1 partition sums (f32 psum acc)
                    dsum = psDB.tile([1, 512], dt.float32, name="dsum", tag="db")
                    nc.tensor.matmul(dsum[:], ones_col[:], dacc[:, 0:512], start=True,
                                     stop=(qb == 0), skip_group_check=True)
                    if qb > 0:
                        nc.tensor.matmul(dsum[:], ones_col[:], dacc[:, 512:1024],
                                         start=False, stop=True, skip_group_check=True)

                    def fin(h=h, ops=ops, dsum=dsum):
                        dsum_sb = denp.tile([1, 512], dt.float32, name="dsum_sb")
                        nc.vector.tensor_copy(dsum_sb[:], dsum[:])
                        rec1 = denp.tile([1, 512], dt.bfloat16, name="rec1")
                        with nc.allow_low_precision(reason="1/den broadcast in bf16"):
                            nc.vector.reciprocal(rec1[:], dsum_sb[:])
                        # broadcast 1/den across partitions: rank-1 PE matmul
                        # (keeps Pool free -- the collective blocks that queue)
                        dbc = psDB.tile([128, 512], dt.float32, name="dbc", tag="db")
                        nc.tensor.matmul(dbc[:], ones_row[:], rec1[:],
                                         start=True, stop=True, skip_group_check=True)
                        rbc = denp.tile([128, 512], dt.bfloat16, name="rbc")
                        nc.scalar.copy(rbc[:], dbc[:])
                        oT = oTp.tile([128, 512], dt.bfloat16, name="oT")
                        nc.vector.tensor_mul(oT[:], ops[:], rbc[:])
                        if "C" in stages:
                            nc.sync.dma_start(agins[qb][h * 128:(h + 1) * 128, :], oT[:])

                    if h < NQH - 1:
                        finish[0] = fin
                    else:
                        fin()  # last head: finish immediately so the AG fires asap
                while dq:
                    dq.pop(0)(psW)

            # ---------------- main loop ----------------
            import os as _os
            d_fifo = []   # FIFO of quarter wo-GEMM work; consumed DLAG quarters later
            DLAG = int(_os.environ.get("DLAG", "4"))
            for rep in range(reps):
                for qb in range(NSB):
                    emit_a_block(qb)
                    if rep == 0 and qb == 0:
                        for g in range(NG):
                            nc.scalar.dma_start(
                                wo_g[g][:].rearrange("p (c m) -> p c m", c=4),
                                wo_e.ap()[g * 512:(g + 1) * 512, :].rearrange(
                                    "(c p) m -> p c m", p=128))
                    if "B" not in stages:
                        continue
                    nxt = qb + 1 if qb + 1 < NSB else (0 if rep + 1 < reps else None)
                    with (
                        tc.tile_pool(name="psS", bufs=2, space="PSUM") as psS,
                        tc.tile_pool(name="psO", bufs=1, space="PSUM") as psO,
                        tc.tile_pool(name="psDB", bufs=2, space="PSUM") as psDB,
                        tc.tile_pool(name="psW", bufs=1, space="PSUM") as psW,
                    ):
                        dq = d_fifo.pop(0) if len(d_fifo) >= DLAG else []
                        emit_attn_block(qb, dq, nxt, psS, psO, psDB, psW)
                        emit_ag(qb)
                        if do_d:
                            d_fifo.append(make_d_quarter(qb))

            # tail: flush the last quarter's wo GEMM
            if d_fifo:
                with tc.tile_pool(name="psWt", bufs=2, space="PSUM") as psWt:
                    for work in d_fifo:
                        for ch in work:
                            ch(psWt)

    nc.compile()
    return nc


def _prep_inputs(x, wq, wk, wv, wo):
    """Host-side sharding/layout prep. Returns per-core in_maps."""
    x2 = np.asarray(x, dtype=np.float32).reshape(SEQ, DIM)
    xT = np.ascontiguousarray(x2.T).astype(BF16)

    perm_head = np.concatenate([np.arange(0, HD, 2), np.arange(1, HD, 2)])
    qperm = np.concatenate([g * HD + perm_head for g in range(32)])
    kperm = np.concatenate([g * HD + perm_head for g in range(8)])
    wq_p = np.asarray(wq, dtype=np.float32)[:, qperm].astype(BF16)
    wk_p = np.asarray(wk, dtype=np.float32)[:, kperm].astype(BF16)
    wv_b = np.asarray(wv, dtype=np.float32).astype(BF16)
    wo_b = np.asarray(wo, dtype=np.float32).astype(BF16)

    inv_freq = 1.0 / (10000.0 ** (np.arange(0, HD, 2, dtype=np.float64) / HD))
    ang = inv_freq[:, None] * np.arange(SEQ, dtype=np.float64)[None, :]
    cosd = np.cos(ang)
    sind = np.sin(ang)
    cs = np.concatenate([cosd, cosd, sind, sind]).astype(BF16)

    in_maps = []
    for i in range(N_CORES):
        in_maps.append({
            "xT": xT,
            "wq": np.ascontiguousarray(wq_p[:, i * QCOLS:(i + 1) * QCOLS]),
            "wk": np.ascontiguousarray(wk_p[:, i * HD:(i + 1) * HD]),
            "wv": np.ascontiguousarray(wv_b[:, i * HD:(i + 1) * HD]),
            "wo": np.ascontiguousarray(wo_b[:, i * QCOLS:(i + 1) * QCOLS]),
            "cs": cs,
        })
    return in_maps


def _get_nc(reps: int = 1, stages: str = "ABCD"):
    key = ("nc", reps, stages)
    if key not in _cache:
        _cache[key] = _build_nc(reps, stages)
    return _cache[key]


def kernel(x, wq, wk, wv, wo, start_pos=0, **_ignored):
    from concourse.bass_utils import run_bass_kernel_spmd

    nc = _get_nc()
    in_maps = _prep_inputs(x, wq, wk, wv, wo)
    res = run_bass_kernel_spmd(nc, in_maps, core_ids=list(range(N_CORES)))
    out = np.concatenate([res.results[i]["out"] for i in range(N_CORES)], axis=1)
    return out.reshape(1, SEQ, DIM).astype(np.float32)
